# revision 21
# baseline (speedup 1.0000x reference)
"""Trainium2 Bass kernel for nn_CenMoEDynamicsModel (MoE routing).

Contract: kernel(**inputs) takes FULL unsharded numpy inputs and returns the
FULL [64, 2048, 128] f32 output. Data-parallel over B across 8 NeuronCores
(8 batches/core), expert weights replicated (collectives measured ~67us
fixed cost here - too slow for expert-parallelism at this size).

Math (per batch b):
  x = [z|a]                       [N, D]     D = 192
  w = x @ phi                     [N, E]     E = 16
  dispatch = softmax_n(w); xin = dispatch^T @ x          [E, D]
  h = mish(LN(xin@W1+b1)); h = mish(LN(h@W2+b2)); EO = h@W3+b3   [E, DZ]
  combine = softmax_e(w); out = combine @ EO             [N, DZ]

Design notes (PE-instruction-count driven; ~160ns/instruction fixed cost
dominates over cycles at these shapes):
  - all DMA'd data bf16 with host-pre-arranged dense layouts (>=2KB
    contiguous per partition line); LN/mish internals + output f32.
  - softmaxes via unshifted exp; the dispatch denominator rides as two ones
    columns baked into x => an s row in xin; each pre-LN row is then
    uniformly scaled by s (b1 rides the s row of augmented W1), so LayerNorm
    scale-invariance makes explicit 1/s normalization unnecessary.
  - exp(w) is e-major per batch [E, N] (combine lhsT); token-major copies
    for the xin matmuls come from transposes of 4-batch stacks ([64, 128]
    chunks => 32 transposes/core instead of 128), expressed as regular bf16
    matmuls against identity slices (exact for bf16, cheap moving dim).
  - xin per (batch, chunk): one [16, 194] matmul (ec slice stationary,
    x chunk moving); 2 identity-matmul transposes per batch produce the
    [d, (e, b)] pack for layer 1.
  - combine denominator via ones-columns carried in the expert outputs,
    normalized during PSUM evacuation.
"""

import sys

import numpy as np

sys.path.insert(0, "/opt/trn_rl_repo")

from contextlib import ExitStack

import concourse.bass as bass
import concourse.tile as tile
from concourse import mybir

F32 = mybir.dt.float32
BF = mybir.dt.bfloat16
AF = mybir.ActivationFunctionType

LN_EPS = 1e-5
NCORES = 8


def _split_drain_and_barrier(self, tick_clock, wait_clock):
    """Replacement for TileContext._drain_and_barrier.

    The stock version attaches every outstanding semaphore wait to ONE tail
    Drain instruction; this walrus build's codegen rejects Drains with more
    than a couple of sync waits ("Too many sync wait commands"). Emit one
    single-wait Drain per logical proc instead (the SP queue executes them in
    order, so the final bare drain still happens after everything finished).
    """
    from concourse.vector_clock import ScopedClock, VectorClock

    nc = self.nc
    gc = tick_clock.global_clock
    n = len(gc)
    for i in range(n):
        t = gc[i]
        if t <= 0:
            continue
        v = VectorClock([0] * n)
        v.require_at_least(i, t)
        d = nc.sync.drain()
        wait_clock.add_sem_waits(d.ins, ScopedClock({None: v}))
    nc.sync.drain()
    nc.all_engine_barrier()
    assert self.sems is not None
    popped = nc._tile_sem_poison_stack.pop()
    assert popped is self._sem_poison
    nc.clear_and_free_semaphores(list(self.sems.allocated().values()))
    nc.all_engine_barrier()


tile.TileContext._drain_and_barrier = _split_drain_and_barrier

# This walrus build rejects instructions carrying more than a couple of sync
# waits ("Too many sync wait commands" in CoreV3 codegen), while Tile freely
# attaches 3+. Split excess waits onto NoOp carrier instructions (same engine
# queue, executed in order => semantics preserved) at BIR-serialization time.
_MAX_WAITS = 1


def _split_waits_json(bir: bytes) -> bytes:
    import orjson

    m = orjson.loads(bir)
    changed = False
    ctr = 0
    for f in m.get("functions", []):
        for b in f.get("blocks", []):
            out = []
            for i in b.get("instructions", []):
                si = i.get("sync_info")
                ow = (si or {}).get("on_wait") or []
                if len(ow) > _MAX_WAITS:
                    head = ow[: -_MAX_WAITS]
                    for j in range(0, len(head), _MAX_WAITS):
                        ctr += 1
                        out.append(
                            {
                                "debug": i.get("debug", 0),
                                "engine": i["engine"],
                                "ins": [],
                                "outs": [],
                                "name": f"{i['name']}-wsplit{ctr}",
                                "opcode": "NoOp",
                                "sync_info": {
                                    "on_wait": head[j : j + _MAX_WAITS],
                                    "on_update": [],
                                },
                            }
                        )
                    si["on_wait"] = ow[-_MAX_WAITS:]
                    changed = True
                out.append(i)
            b["instructions"] = out
    return orjson.dumps(m) if changed else bir


_orig_to_json_bytes = bass.Bass.to_json_bytes


def _patched_to_json_bytes(self):
    return _split_waits_json(_orig_to_json_bytes(self))


bass.Bass.to_json_bytes = _patched_to_json_bytes


def build_nc(BC, N, DZ, DA, E, H1, H2, has_b2, has_b3, has_g1, has_g2):
    """Build the per-core Bass program.

    BC batches per core. Requires DZ == 128, N % 512 == 0, H1 % 128 == 0,
    H2 % 128 == 0, E * BC == 128, E <= 16.
    """
    D = DZ + DA  # 192
    DP = D + 2  # 194 (two ones columns)
    NT = N // 128  # 16
    NC = N // 512  # 4
    EB = E * BC  # 128
    C1 = H1 // 128
    C2 = H2 // 128
    G = 4  # batches per transpose stack
    NG = BC // G
    assert DZ == 128 and DA < 128 and EB == 128 and N % 512 == 0 and E <= 16

    nc = bass.Bass()

    # ---- DRAM tensors: all host-pre-arranged for dense [<=128, F] DMAs ----
    x_d = nc.dram_tensor("x", [BC, 128, NT * DP], BF, kind="ExternalInput")
    xT_d = nc.dram_tensor("xT", [BC, D, N], BF, kind="ExternalInput")
    phi_d = nc.dram_tensor("phi", [D, E], BF, kind="ExternalInput")
    w1h_d = nc.dram_tensor("w1h", [128, E * H1], BF, kind="ExternalInput")
    w1l_d = nc.dram_tensor(
        "w1l", [DP - 128, E * H1], BF, kind="ExternalInput"
    )
    w2_d = nc.dram_tensor("w2", [128, E * C1 * H2], BF, kind="ExternalInput")
    w3_d = nc.dram_tensor("w3", [128, E * C2 * DZ], BF, kind="ExternalInput")
    ident_d = nc.dram_tensor("ident", [128, 128], BF, kind="ExternalInput")
    ones_d = nc.dram_tensor("ones", [128, 128], BF, kind="ExternalInput")
    if has_b2:
        b2_d = nc.dram_tensor("b2", [1, E * H2], BF, kind="ExternalInput")
    if has_b3:
        b3_d = nc.dram_tensor("b3", [1, E * DZ], BF, kind="ExternalInput")
    if has_g1:
        g1_d = nc.dram_tensor("g1r", [EB, H1], F32, kind="ExternalInput")
        be1_d = nc.dram_tensor("be1r", [EB, H1], F32, kind="ExternalInput")
    if has_g2:
        g2_d = nc.dram_tensor("g2r", [EB, H2], F32, kind="ExternalInput")
        be2_d = nc.dram_tensor("be2r", [EB, H2], F32, kind="ExternalInput")
    out_d = nc.dram_tensor(
        "out", [BC, 128, NT * DZ], F32, kind="ExternalOutput"
    )

    with tile.TileContext(nc) as tc, ExitStack() as ctx:
        perm = ctx.enter_context(tc.tile_pool(name="perm", bufs=1))
        ident = perm.tile([128, 128], BF)
        ones_sb = perm.tile([128, 128], BF)
        phi_hi = perm.tile([128, E], BF)
        phi_lo = perm.tile([DA, E], BF)
        nc.sync.dma_start(ident[:], ident_d[:, :])
        nc.sync.dma_start(ones_sb[:], ones_d[:, :])
        nc.sync.dma_start(phi_hi[:], phi_d[0:128, :])
        nc.sync.dma_start(phi_lo[:], phi_d[128:D, :])

        # Weight loads: issued up front on the gpsimd (SWDGE) queue so they
        # stream during phase A.
        w1h_sb = perm.tile([128, E * H1], BF, name="w1h_sb")
        w1l_sb = perm.tile([DP - 128, E * H1], BF, name="w1l_sb")
        w3_sb = perm.tile([128, E * C2 * DZ], BF, name="w3_sb")
        NW2 = 4  # experts per w2 slab
        w2_sb = [
            perm.tile([128, NW2 * C1 * H2], BF, name=f"w2_sb{i}")
            for i in range(E // NW2)
        ]
        nc.gpsimd.dma_start(w1h_sb[:], w1h_d[:, :])
        nc.gpsimd.dma_start(w1l_sb[:], w1l_d[:, :])
        nc.gpsimd.dma_start(w3_sb[:], w3_d[:, :])
        for i in range(E // NW2):
            nc.gpsimd.dma_start(
                w2_sb[i][:], w2_d[:, i * NW2 * C1 * H2 : (i + 1) * NW2 * C1 * H2]
            )
        if has_b2:
            b2sb = perm.tile([1, E * H2], BF)
            nc.gpsimd.dma_start(b2sb[:], b2_d[:, :])
        if has_b3:
            b3sb = perm.tile([1, E * DZ], BF)
            nc.gpsimd.dma_start(b3sb[:], b3_d[:, :])
        g1sb = be1sb = g2sb = be2sb = None
        if has_g1:
            g1sb = perm.tile([EB, H1], F32)
            be1sb = perm.tile([EB, H1], F32)
            nc.gpsimd.dma_start(g1sb[:], g1_d[:, :])
            nc.gpsimd.dma_start(be1sb[:], be1_d[:, :])
        if has_g2:
            g2sb = perm.tile([EB, H2], F32)
            be2sb = perm.tile([EB, H2], F32)
            nc.gpsimd.dma_start(g2sb[:], g2_d[:, :])
            nc.gpsimd.dma_start(be2sb[:], be2_d[:, :])

        # exp(w) e-major per batch (combine lhsT; partitions 0..E-1)
        expCT = [
            perm.tile([E, N], BF, tag=f"expCT{b}", name=f"expCT{b}")
            for b in range(BC)
        ]
        # 4-batch stacks for the shared transposes (partitions 0..G*E-1)
        expG = [
            perm.tile([G * E, N], BF, tag=f"expG{g}", name=f"expG{g}")
            for g in range(NG)
        ]
        # token-major exp(w) per stack: [tok, (t, b in stack, e)]
        ecG = [
            perm.tile([128, NT * G * E], BF, tag=f"ecG{g}", name=f"ecG{g}")
            for g in range(NG)
        ]
        # xin pack [d, (e, b)]; lo rows DA..DA+1 hold s_e
        xin_hi = perm.tile([128, EB], BF)
        xin_lo = perm.tile([DP - 128, EB], BF)
        # expert outputs per batch [E, DZ]
        eo = [
            perm.tile([E, DZ], BF, tag=f"eo{b}", name=f"eo{b}")
            for b in range(BC)
        ]
        eps_col = perm.tile([128, 1], F32)
        nc.vector.memset(eps_col[:], LN_EPS)

        # ---------------- Phase A: routing + xin ----------------
        with tc.tile_pool(name="pa", bufs=3) as pa, tc.tile_pool(
            name="pa2", bufs=2
        ) as pa2, tc.tile_pool(name="pa_ps_w", bufs=2, space="PSUM") as ppw, tc.tile_pool(
            name="pa_ps_tr", bufs=2, space="PSUM"
        ) as ppt, tc.tile_pool(
            name="pa_ps_xin", bufs=2, space="PSUM"
        ) as ppx, tc.tile_pool(name="pa_ps_xt", bufs=1, space="PSUM") as ppxt:
            for g in range(NG):
                xvs = {}
                for bg in range(G):
                    b = g * G + bg
                    xT_hi = pa.tile([128, N], BF, tag="xth")
                    xT_lo = pa.tile([DA, N], BF, tag="xtl")
                    # split loads so the first w matmul starts earlier
                    nc.sync.dma_start(xT_hi[:, 0 : N // 2], xT_d[b, 0:128, 0 : N // 2])
                    nc.sync.dma_start(xT_hi[:, N // 2 : N], xT_d[b, 0:128, N // 2 : N])
                    nc.sync.dma_start(xT_lo[:], xT_d[b, 128:D, :])
                    # xv on the Act HWDGE queue: parallel to the sync queue's
                    # xT stream (two HWDGE rings drain independently)
                    xv_t = pa.tile([128, NT * DP], BF, tag="x")
                    nc.scalar.dma_start(xv_t[:], x_d[b])
                    xvs[bg] = xv_t
                    # w^T = phi^T @ xT in 512-col chunks, exp -> expCT[b]
                    for c in range(NC):
                        wps = ppw.tile([E, 512], F32, tag="wt")
                        sl = slice(512 * c, 512 * (c + 1))
                        nc.tensor.matmul(
                            wps[:], phi_hi[:], xT_hi[:, sl], start=True, stop=False
                        )
                        nc.tensor.matmul(
                            wps[:], phi_lo[:], xT_lo[:, sl], start=False, stop=True
                        )
                        nc.scalar.activation(expCT[b][:, sl], wps[:], AF.Exp)
                    # partition-move into the stack tile (rows E*bg..)
                    nc.gpsimd.dma_start(
                        expG[g][E * bg : E * (bg + 1), :], expCT[b][:]
                    )
                # shared transposes: [G*E, 128] chunks -> [128, G*E]
                egv = expG[g][:].rearrange("q (p t) -> q t p", t=NT)
                for t2 in range(NT // 2):
                    trp = ppt.tile([128, 2 * G * E], F32, tag="trp")
                    for k in range(2):
                        nc.tensor.matmul(
                            trp[:, k * G * E : (k + 1) * G * E],
                            egv[:, 2 * t2 + k, :],
                            ident[0 : G * E, 0 : G * E],
                            start=True,
                            stop=True,
                        )
                    if t2 % 2 == 0:
                        nc.vector.tensor_copy(
                            ecG[g][:, 2 * G * E * t2 : 2 * G * E * (t2 + 1)],
                            trp[:],
                        )
                    else:
                        nc.scalar.copy(
                            ecG[g][:, 2 * G * E * t2 : 2 * G * E * (t2 + 1)],
                            trp[:],
                        )
                # xin per batch in this stack
                for bg in range(G):
                    b = g * G + bg
                    xv = xvs[bg]
                    xps = ppx.tile([E, DP], F32, tag="xps")
                    for t in range(NT):
                        nc.tensor.matmul(
                            xps[:],
                            ecG[g][:, G * E * t + E * bg : G * E * t + E * (bg + 1)],
                            xv[:].rearrange("p (t c) -> p t c", c=DP)[:, t, :],
                            start=(t == 0),
                            stop=(t == NT - 1),
                        )
                    xin_sb = pa2.tile([E, DP], BF, tag="xsb")
                    nc.scalar.copy(xin_sb[:], xps[:])
                    # transpose into the L1 pack layout [d, (e, b)]
                    xth = ppxt.tile([128, E], F32, tag="xh")
                    xtl = ppxt.tile([DP - 128, E], F32, tag="xl")
                    nc.tensor.matmul(
                        xth[:], xin_sb[:, 0:128], ident[0:E, 0:E], start=True, stop=True
                    )
                    nc.tensor.matmul(
                        xtl[:],
                        xin_sb[:, 128:DP],
                        ident[0:E, 0:E],
                        start=True,
                        stop=True,
                    )
                    xhv = xin_hi[:].rearrange("p (e w) -> p e w", w=BC)
                    xlv = xin_lo[:].rearrange("p (e w) -> p e w", w=BC)
                    nc.vector.tensor_copy(xhv[:, :, b], xth[:])
                    nc.vector.tensor_copy(xlv[:, :, b], xtl[:])

        # ---------------- MLP phase (packed over (e, b) rows) ----------------
        def ln_mish(hs, pool, H, gr, ber):
            """LayerNorm + mish of SBUF [EB, H] f32 -> bf16."""
            s1 = pool.tile([EB, 1], F32, tag="s1")
            nc.vector.reduce_sum(s1[:], hs, axis=mybir.AxisListType.X)
            mean = pool.tile([EB, 1], F32, tag="mean")
            nc.scalar.mul(mean[:], s1[:], 1.0 / H)
            xc = pool.tile([EB, H], F32, tag="xc")
            nc.vector.tensor_scalar_sub(xc[:], hs, mean[:])
            sq = pool.tile([EB, H], F32, tag="sq")
            var = pool.tile([EB, 1], F32, tag="var")
            nc.scalar.activation(sq[:], xc[:], AF.Square, accum_out=var[:])
            std = pool.tile([EB, 1], F32, tag="std")
            nc.scalar.activation(
                std[:], var[:], AF.Sqrt, bias=eps_col[0:EB, :], scale=1.0 / H
            )
            rstd = pool.tile([EB, 1], F32, tag="rstd")
            nc.vector.reciprocal(rstd[:], std[:])
            xn = pool.tile([EB, H], F32, tag="xn")
            nc.vector.tensor_scalar_mul(xn[:], xc[:], rstd[:])
            if gr is not None:
                xg = pool.tile([EB, H], F32, tag="xg")
                nc.vector.tensor_mul(xg[:], xn[:], gr)
                xn = pool.tile([EB, H], F32, tag="xb")
                nc.vector.tensor_add(xn[:], xg[:], ber)
            # mish(x) = x * tanh(ln(1 + e^x))
            ex = pool.tile([EB, H], F32, tag="ex")
            nc.scalar.activation(ex[:], xn[:], AF.Exp)
            sp = pool.tile([EB, H], F32, tag="sp")
            nc.scalar.activation(sp[:], ex[:], AF.Ln, bias=1.0)
            th = pool.tile([EB, H], F32, tag="th")
            nc.scalar.activation(th[:], sp[:], AF.Tanh)
            hm = pool.tile([EB, H], BF, tag="hm")
            nc.vector.tensor_mul(hm[:], xn[:], th[:])
            return hm

        def transpose_pack(hm, pool, ppool, H, name):
            """[EB, H] bf16 -> hT [128, (H//128)*EB] via identity matmuls."""
            hT = pool.tile([128, (H // 128) * EB], BF, tag=name, name=name)
            for c in range(H // 128):
                ptp = ppool.tile([128, EB], F32, tag="mtr")
                nc.tensor.matmul(
                    ptp[:],
                    hm[:, 128 * c : 128 * (c + 1)],
                    ident[:, 0:EB],
                    start=True,
                    stop=True,
                )
                if c % 2 == 0:
                    nc.vector.tensor_copy(hT[:, c * EB : (c + 1) * EB], ptp[:])
                else:
                    nc.scalar.copy(hT[:, c * EB : (c + 1) * EB], ptp[:])
            return hT

        with tc.tile_pool(name="pm", bufs=1) as pm, tc.tile_pool(
            name="pm_st", bufs=3
        ) as pst, tc.tile_pool(name="pm_ps", bufs=3, space="PSUM") as pmps, tc.tile_pool(
            name="pm_ps_tr", bufs=2, space="PSUM"
        ) as pmpst, tc.tile_pool(name="pm_ps_eo", bufs=1, space="PSUM") as pmpse:
            w1hv = w1h_sb[:].rearrange("p (e h) -> p e h", e=E)
            w1lv = w1l_sb[:].rearrange("p (e h) -> p e h", e=E)
            h1_all = pm.tile([EB, H1], F32, tag="h1_all", name="h1_all")
            for e in range(E):
                hp = pmps.tile([BC, H1], F32, tag="hp")
                nc.tensor.matmul(
                    hp[:],
                    xin_hi[:, e * BC : (e + 1) * BC],
                    w1hv[:, e, :],
                    start=True,
                    stop=False,
                )
                nc.tensor.matmul(
                    hp[:],
                    xin_lo[:, e * BC : (e + 1) * BC],
                    w1lv[:, e, :],
                    start=False,
                    stop=True,
                )
                hst = pst.tile([BC, H1], F32, tag="hst")
                if e % 2 == 0:
                    nc.vector.tensor_copy(hst[:], hp[:])
                else:
                    nc.scalar.copy(hst[:], hp[:])
                eng = nc.gpsimd if e % 2 == 0 else nc.sync
                eng.dma_start(h1_all[e * BC : (e + 1) * BC, :], hst[:])
            h1m = ln_mish(
                h1_all[:],
                pm,
                H1,
                g1sb[:] if has_g1 else None,
                be1sb[:] if has_g1 else None,
            )
            h1T = transpose_pack(h1m, pm, pmpst, H1, "h1T")

            h2_all = pm.tile([EB, H2], F32, tag="h2_all", name="h2_all")
            for e in range(E):
                w2v = w2_sb[e // NW2][:].rearrange(
                    "p (q c h) -> p q c h", q=NW2, c=C1
                )
                hp = pmps.tile([BC, H2], F32, tag="hp")
                for c1 in range(C1):
                    nc.tensor.matmul(
                        hp[:],
                        h1T[:, c1 * EB + e * BC : c1 * EB + (e + 1) * BC],
                        w2v[:, e % NW2, c1, :],
                        start=(c1 == 0),
                        stop=(c1 == C1 - 1 and not has_b2),
                    )
                if has_b2:
                    nc.tensor.matmul(
                        hp[:],
                        ones_sb[0:1, 0:BC],
                        b2sb[0:1, e * H2 : (e + 1) * H2],
                        start=False,
                        stop=True,
                    )
                hst = pst.tile([BC, H2], F32, tag="hst")
                if e % 2 == 0:
                    nc.vector.tensor_copy(hst[:], hp[:])
                else:
                    nc.scalar.copy(hst[:], hp[:])
                eng = nc.gpsimd if e % 2 == 0 else nc.sync
                eng.dma_start(h2_all[e * BC : (e + 1) * BC, :], hst[:])
            h2m = ln_mish(
                h2_all[:],
                pm,
                H2,
                g2sb[:] if has_g2 else None,
                be2sb[:] if has_g2 else None,
            )
            h2T = transpose_pack(h2m, pm, pmpst, H2, "h2T")

            # Layer 3 -> eops [DZ, (e, b)]
            w3v = w3_sb[:].rearrange("p (e c d) -> p e c d", e=E, c=C2)
            eops = pmpse.tile([128, EB], F32, tag="eot")
            for e in range(E):
                for c in range(C2):
                    nc.tensor.matmul(
                        eops[0:DZ, e * BC : (e + 1) * BC],
                        w3v[:, e, c, :],
                        h2T[:, c * EB + e * BC : c * EB + (e + 1) * BC],
                        start=(c == 0),
                        stop=(c == C2 - 1 and not has_b3),
                    )
                if has_b3:
                    nc.tensor.matmul(
                        eops[0:DZ, e * BC : (e + 1) * BC],
                        b3sb[0:1, e * DZ : (e + 1) * DZ],
                        ones_sb[0:1, 0:BC],
                        start=False,
                        stop=True,
                    )
            eot_sb = pm.tile([128, EB], BF, tag="eot_sb")
            nc.vector.tensor_copy(eot_sb[:], eops[:])
            # regroup into per-batch [E, DZ+2] via identity matmuls
            eotv = eot_sb[:].rearrange("p (e w) -> p w e", w=BC)
            for b in range(BC):
                peo = pmpst.tile([E, DZ], F32, tag="peo")
                nc.tensor.matmul(
                    peo[:], eotv[:, b, :], ident[:, 0:DZ], start=True, stop=True
                )
                nc.vector.tensor_copy(eo[b][:, 0:DZ], peo[:])

        # ---------------- Combine phase ----------------
        with tc.tile_pool(name="pc", bufs=4) as pc, tc.tile_pool(
            name="pc_st", bufs=2
        ) as pcst, tc.tile_pool(name="pc_ps", bufs=6, space="PSUM") as pcps:
            for b in range(BC):
                osb = pcst.tile([128, NT * DZ], F32, tag="osb")
                ov = osb[:].rearrange("p (t d) -> p t d", d=DZ)
                ecv = expCT[b][:].rearrange("e (p t) -> e t p", t=NT)
                # combine softmax denominators for all chunks at once, from
                # the token-major copy of the same bf16 exp values
                ecv4 = ecG[b // G][:].rearrange(
                    "p (t w e) -> p t w e", w=G, e=E
                )
                sn = pc.tile([128, NT], F32, tag="sn")
                nc.vector.reduce_sum(
                    sn[:], ecv4[:, :, b % G, :], axis=mybir.AxisListType.X
                )
                rna = pc.tile([128, NT], F32, tag="rna")
                nc.vector.reciprocal(rna[:], sn[:])
                for t in range(NT):
                    ops = pcps.tile([128, DZ], F32, tag="o")
                    nc.tensor.matmul(
                        ops[:], ecv[:, t, :], eo[b][:, :], start=True, stop=True
                    )
                    if t % 2 == 0:
                        nc.vector.tensor_scalar_mul(
                            ov[:, t, :], ops[:], rna[:, t : t + 1]
                        )
                    else:
                        nc.scalar.mul(ov[:, t, :], ops[:], rna[:, t : t + 1])
                nc.sync.dma_start(out_d[b], osb[:])
    return nc


# ---------------------------------------------------------------------------
# Host wrapper
# ---------------------------------------------------------------------------

_CACHE = {}


def _get_nc(key, *args):
    if key not in _CACHE:
        _CACHE[key] = build_nc(*args)
    return _CACHE[key]


def _prepare(z, a, phi, W1, b1, g1, be1, W2, b2, g2, be2, W3, b3):
    """Build (cached) the Bass program and per-core input maps."""
    import ml_dtypes

    BFNP = ml_dtypes.bfloat16

    z = np.asarray(z, np.float32)
    a = np.asarray(a, np.float32)
    phi = np.asarray(phi, np.float32)
    W1 = np.asarray(W1, np.float32)
    b1 = np.asarray(b1, np.float32)
    g1 = np.asarray(g1, np.float32)
    be1 = np.asarray(be1, np.float32)
    W2 = np.asarray(W2, np.float32)
    b2 = np.asarray(b2, np.float32)
    g2 = np.asarray(g2, np.float32)
    be2 = np.asarray(be2, np.float32)
    W3 = np.asarray(W3, np.float32)
    b3 = np.asarray(b3, np.float32)

    B, N, DZ = z.shape
    DA = a.shape[2]
    D = DZ + DA
    DP = D + 2
    E = W1.shape[0]
    H1 = W1.shape[2]
    H2 = W2.shape[2]
    C1 = H1 // 128
    C2 = H2 // 128
    BC = B // NCORES
    NT = N // 128

    has_b2 = bool(np.any(b2))
    has_b3 = bool(np.any(b3))
    has_g1 = not (np.all(g1 == 1.0) and np.all(be1 == 0.0))
    has_g2 = not (np.all(g2 == 1.0) and np.all(be2 == 0.0))

    key = (BC, N, DZ, DA, E, H1, H2, has_b2, has_b3, has_g1, has_g2)
    nc = _get_nc(key, *key)

    x_full = np.empty((B, N, DP), np.float32)
    x_full[:, :, 0:DZ] = z
    x_full[:, :, DZ:D] = a
    x_full[:, :, D:DP] = 1.0
    x_bf = x_full.astype(BFNP).reshape(B, 128, NT * DP)
    xT_bf = np.ascontiguousarray(
        x_full[:, :, 0:D].transpose(0, 2, 1)
    ).astype(BFNP)
    phi_bf = np.ascontiguousarray(phi.reshape(D, E)).astype(BFNP)
    w1aug = np.zeros((E, DP, H1), np.float32)
    w1aug[:, 0:D, :] = W1
    w1aug[:, D, :] = b1
    w1h = np.ascontiguousarray(
        w1aug[:, 0:128, :].transpose(1, 0, 2).reshape(128, E * H1)
    ).astype(BFNP)
    w1l = np.ascontiguousarray(
        w1aug[:, 128:DP, :].transpose(1, 0, 2).reshape(DP - 128, E * H1)
    ).astype(BFNP)
    w2p = np.ascontiguousarray(
        W2.reshape(E, C1, 128, H2).transpose(2, 0, 1, 3).reshape(128, E * C1 * H2)
    ).astype(BFNP)
    w3p = np.ascontiguousarray(
        W3.reshape(E, C2, 128, DZ).transpose(2, 0, 1, 3).reshape(128, E * C2 * DZ)
    ).astype(BFNP)
    ident_np = np.eye(128, dtype=np.float32).astype(BFNP)
    ones_np = np.ones((128, 128), np.float32).astype(BFNP)

    in_maps = []
    for i in range(NCORES):
        m = {
            "x": np.ascontiguousarray(x_bf[i * BC : (i + 1) * BC]),
            "xT": np.ascontiguousarray(xT_bf[i * BC : (i + 1) * BC]),
            "phi": phi_bf,
            "w1h": w1h,
            "w1l": w1l,
            "w2": w2p,
            "w3": w3p,
            "ident": ident_np,
            "ones": ones_np,
        }
        if has_b2:
            m["b2"] = b2.reshape(1, E * H2).astype(BFNP)
        if has_b3:
            m["b3"] = b3.reshape(1, E * DZ).astype(BFNP)
        if has_g1:
            m["g1r"] = np.ascontiguousarray(np.repeat(g1, BC, axis=0))
            m["be1r"] = np.ascontiguousarray(np.repeat(be1, BC, axis=0))
        if has_g2:
            m["g2r"] = np.ascontiguousarray(np.repeat(g2, BC, axis=0))
            m["be2r"] = np.ascontiguousarray(np.repeat(be2, BC, axis=0))
        in_maps.append(m)
    return nc, in_maps


def kernel(**inputs):
    nc, in_maps = _prepare(**inputs)

    from concourse.bass_utils import run_bass_kernel_spmd

    res = run_bass_kernel_spmd(nc, in_maps, list(range(NCORES)))
    B = inputs["z"].shape[0]
    N = inputs["z"].shape[1]
    DZ = inputs["z"].shape[2]
    return np.concatenate([r["out"] for r in res.results], axis=0).reshape(
        B, N, DZ
    )


# revision 26
# speedup vs baseline: 1.0308x; 1.0308x over previous
"""Trainium2 Bass kernel for nn_CenMoEDynamicsModel (MoE routing).

Contract: kernel(**inputs) takes FULL unsharded numpy inputs and returns the
FULL [64, 2048, 128] f32 output. Data-parallel over B across 8 NeuronCores
(8 batches/core), expert weights replicated (collectives measured ~67us
fixed cost here - too slow for expert-parallelism at this size).

Math (per batch b):
  x = [z|a]                       [N, D]     D = 192
  w = x @ phi                     [N, E]     E = 16
  dispatch = softmax_n(w); xin = dispatch^T @ x          [E, D]
  h = mish(LN(xin@W1+b1)); h = mish(LN(h@W2+b2)); EO = h@W3+b3   [E, DZ]
  combine = softmax_e(w); out = combine @ EO             [N, DZ]

Design notes (PE-instruction-count driven; ~160ns/instruction fixed cost
dominates over cycles at these shapes):
  - all DMA'd data bf16 with host-pre-arranged dense layouts (>=2KB
    contiguous per partition line); LN/mish internals + output f32.
  - softmaxes via unshifted exp; the dispatch denominator rides as two ones
    columns baked into x => an s row in xin; each pre-LN row is then
    uniformly scaled by s (b1 rides the s row of augmented W1), so LayerNorm
    scale-invariance makes explicit 1/s normalization unnecessary.
  - exp(w) is e-major per batch [E, N] (combine lhsT); token-major copies
    for the xin matmuls come from transposes of 4-batch stacks ([64, 128]
    chunks => 32 transposes/core instead of 128), expressed as regular bf16
    matmuls against identity slices (exact for bf16, cheap moving dim).
  - xin per (batch, chunk): one [16, 194] matmul (ec slice stationary,
    x chunk moving); 2 identity-matmul transposes per batch produce the
    [d, (e, b)] pack for layer 1.
  - combine denominator via ones-columns carried in the expert outputs,
    normalized during PSUM evacuation.
"""

import sys

import numpy as np

sys.path.insert(0, "/opt/trn_rl_repo")

from contextlib import ExitStack

import concourse.bass as bass
import concourse.tile as tile
from concourse import mybir

F32 = mybir.dt.float32
BF = mybir.dt.bfloat16
AF = mybir.ActivationFunctionType

LN_EPS = 1e-5
NCORES = 8


def _split_drain_and_barrier(self, tick_clock, wait_clock):
    """Replacement for TileContext._drain_and_barrier.

    The stock version attaches every outstanding semaphore wait to ONE tail
    Drain instruction; this walrus build's codegen rejects Drains with more
    than a couple of sync waits ("Too many sync wait commands"). Emit one
    single-wait Drain per logical proc instead (the SP queue executes them in
    order, so the final bare drain still happens after everything finished).
    """
    from concourse.vector_clock import ScopedClock, VectorClock

    nc = self.nc
    gc = tick_clock.global_clock
    n = len(gc)
    for i in range(n):
        t = gc[i]
        if t <= 0:
            continue
        v = VectorClock([0] * n)
        v.require_at_least(i, t)
        d = nc.sync.drain()
        wait_clock.add_sem_waits(d.ins, ScopedClock({None: v}))
    nc.sync.drain()
    nc.all_engine_barrier()
    assert self.sems is not None
    popped = nc._tile_sem_poison_stack.pop()
    assert popped is self._sem_poison
    nc.clear_and_free_semaphores(list(self.sems.allocated().values()))
    nc.all_engine_barrier()


tile.TileContext._drain_and_barrier = _split_drain_and_barrier

# This walrus build rejects instructions carrying more than a couple of sync
# waits ("Too many sync wait commands" in CoreV3 codegen), while Tile freely
# attaches 3+. Split excess waits onto NoOp carrier instructions (same engine
# queue, executed in order => semantics preserved) at BIR-serialization time.
_MAX_WAITS = 1


def _split_waits_json(bir: bytes) -> bytes:
    import orjson

    m = orjson.loads(bir)
    changed = False
    ctr = 0
    for f in m.get("functions", []):
        for b in f.get("blocks", []):
            out = []
            for i in b.get("instructions", []):
                si = i.get("sync_info")
                ow = (si or {}).get("on_wait") or []
                if len(ow) > _MAX_WAITS:
                    head = ow[: -_MAX_WAITS]
                    for j in range(0, len(head), _MAX_WAITS):
                        ctr += 1
                        out.append(
                            {
                                "debug": i.get("debug", 0),
                                "engine": i["engine"],
                                "ins": [],
                                "outs": [],
                                "name": f"{i['name']}-wsplit{ctr}",
                                "opcode": "NoOp",
                                "sync_info": {
                                    "on_wait": head[j : j + _MAX_WAITS],
                                    "on_update": [],
                                },
                            }
                        )
                    si["on_wait"] = ow[-_MAX_WAITS:]
                    changed = True
                out.append(i)
            b["instructions"] = out
    return orjson.dumps(m) if changed else bir


_orig_to_json_bytes = bass.Bass.to_json_bytes


def _patched_to_json_bytes(self):
    return _split_waits_json(_orig_to_json_bytes(self))


bass.Bass.to_json_bytes = _patched_to_json_bytes




def build_nc(BC, N, DZ, DA, E, H1, H2, has_b2, has_b3, has_g1, has_g2):
    """Build the per-core Bass program.

    BC batches per core. Requires DZ == 128, N % 512 == 0, H1 % 128 == 0,
    H2 % 128 == 0, E * BC == 128, E <= 16.
    """
    D = DZ + DA  # 192
    DP = D + 2  # 194 (two ones columns)
    NT = N // 128  # 16
    NC = N // 512  # 4
    EB = E * BC  # 128
    C1 = H1 // 128
    C2 = H2 // 128
    G = 4  # batches per transpose stack
    NG = BC // G
    assert DZ == 128 and DA < 128 and EB == 128 and N % 512 == 0 and E <= 16

    nc = bass.Bass()

    # ---- DRAM tensors: all host-pre-arranged for dense [<=128, F] DMAs ----
    x_d = nc.dram_tensor("x", [BC, 128, NT * DP], BF, kind="ExternalInput")
    xT_d = nc.dram_tensor("xT", [BC, D, N], BF, kind="ExternalInput")
    phi_d = nc.dram_tensor("phi", [D, E], BF, kind="ExternalInput")
    w1h_d = nc.dram_tensor("w1h", [128, E * H1], BF, kind="ExternalInput")
    w1l_d = nc.dram_tensor(
        "w1l", [DP - 128, E * H1], BF, kind="ExternalInput"
    )
    w2_d = nc.dram_tensor("w2", [128, E * C1 * H2], BF, kind="ExternalInput")
    w3_d = nc.dram_tensor("w3", [128, E * C2 * DZ], BF, kind="ExternalInput")
    ident_d = nc.dram_tensor("ident", [128, 128], BF, kind="ExternalInput")
    ones_d = nc.dram_tensor("ones", [128, 128], BF, kind="ExternalInput")
    if has_b2:
        b2_d = nc.dram_tensor("b2", [1, E * H2], BF, kind="ExternalInput")
    if has_b3:
        b3_d = nc.dram_tensor("b3", [1, E * DZ], BF, kind="ExternalInput")
    if has_g1:
        g1_d = nc.dram_tensor("g1r", [EB, H1], F32, kind="ExternalInput")
        be1_d = nc.dram_tensor("be1r", [EB, H1], F32, kind="ExternalInput")
    if has_g2:
        g2_d = nc.dram_tensor("g2r", [EB, H2], F32, kind="ExternalInput")
        be2_d = nc.dram_tensor("be2r", [EB, H2], F32, kind="ExternalInput")
    out_d = nc.dram_tensor(
        "out", [BC, 128, NT * DZ], F32, kind="ExternalOutput"
    )

    with tile.TileContext(nc) as tc, ExitStack() as ctx:
        perm = ctx.enter_context(tc.tile_pool(name="perm", bufs=1))
        ident = perm.tile([128, 128], BF)
        ones_sb = perm.tile([128, 128], BF)
        phi_hi = perm.tile([128, E], BF)
        phi_lo = perm.tile([DA, E], BF)
        nc.sync.dma_start(ident[:], ident_d[:, :])
        nc.sync.dma_start(ones_sb[:], ones_d[:, :])
        nc.sync.dma_start(phi_hi[:], phi_d[0:128, :])
        nc.sync.dma_start(phi_lo[:], phi_d[128:D, :])

        # Weight loads: issued up front on the gpsimd (SWDGE) queue so they
        # stream during phase A.
        w1h_sb = perm.tile([128, E * H1], BF, name="w1h_sb")
        w1l_sb = perm.tile([DP - 128, E * H1], BF, name="w1l_sb")
        w3_sb = perm.tile([128, E * C2 * DZ], BF, name="w3_sb")
        NW2 = 4  # experts per w2 slab
        # slabs 0-1 preloaded during phase A; slabs 2-3 allocated in the MLP
        # pool (their SBUF overlaps phase A's x tiles) and stream during L1
        w2_sb = [
            perm.tile([128, NW2 * C1 * H2], BF, name=f"w2_sb{i}")
            for i in range(2)
        ]
        nc.gpsimd.dma_start(w1h_sb[:], w1h_d[:, :])
        nc.gpsimd.dma_start(w1l_sb[:], w1l_d[:, :])
        nc.gpsimd.dma_start(w3_sb[:], w3_d[:, :])
        for i in range(2):
            nc.gpsimd.dma_start(
                w2_sb[i][:], w2_d[:, i * NW2 * C1 * H2 : (i + 1) * NW2 * C1 * H2]
            )
        if has_b2:
            b2sb = perm.tile([1, E * H2], BF)
            nc.gpsimd.dma_start(b2sb[:], b2_d[:, :])
        if has_b3:
            b3sb = perm.tile([1, E * DZ], BF)
            nc.gpsimd.dma_start(b3sb[:], b3_d[:, :])
        g1sb = be1sb = g2sb = be2sb = None
        if has_g1:
            g1sb = perm.tile([EB, H1], F32)
            be1sb = perm.tile([EB, H1], F32)
            nc.gpsimd.dma_start(g1sb[:], g1_d[:, :])
            nc.gpsimd.dma_start(be1sb[:], be1_d[:, :])
        if has_g2:
            g2sb = perm.tile([EB, H2], F32)
            be2sb = perm.tile([EB, H2], F32)
            nc.gpsimd.dma_start(g2sb[:], g2_d[:, :])
            nc.gpsimd.dma_start(be2sb[:], be2_d[:, :])

        # exp(w) e-major per batch (combine lhsT; partitions 0..E-1)
        expCT = [
            perm.tile([E, N], BF, tag=f"expCT{b}", name=f"expCT{b}")
            for b in range(BC)
        ]
        # 4-batch stacks for the shared transposes (partitions 0..G*E-1)
        expG = [
            perm.tile([G * E, N], BF, tag=f"expG{g}", name=f"expG{g}")
            for g in range(NG)
        ]
        # token-major exp(w) per stack: [tok, (t, b in stack, e)]
        ecG = [
            perm.tile([128, NT * G * E], BF, tag=f"ecG{g}", name=f"ecG{g}")
            for g in range(NG)
        ]
        # xin pack [d, (e, b)]; lo rows DA..DA+1 hold s_e
        xin_hi = perm.tile([128, EB], BF)
        xin_lo = perm.tile([DP - 128, EB], BF)
        # expert outputs per batch [E, DZ]
        eo = [
            perm.tile([E, DZ], BF, tag=f"eo{b}", name=f"eo{b}")
            for b in range(BC)
        ]
        eps_col = perm.tile([128, 1], F32)
        nc.vector.memset(eps_col[:], LN_EPS)

        # ---------------- Phase A: routing + xin ----------------
        with tc.tile_pool(name="pa", bufs=8) as pa, tc.tile_pool(
            name="pat", bufs=3
        ) as pat, tc.tile_pool(name="pa2", bufs=2) as pa2, tc.tile_pool(
            name="pa_ps_w", bufs=2, space="PSUM"
        ) as ppw, tc.tile_pool(
            name="pa_ps_tr", bufs=2, space="PSUM"
        ) as ppt, tc.tile_pool(
            name="pa_ps_xin", bufs=2, space="PSUM"
        ) as ppx, tc.tile_pool(name="pa_ps_xt", bufs=1, space="PSUM") as ppxt:
            xvs = {}
            xts = {}
            # all loads first: xT on the sync HWDGE queue, xv on the Act
            # HWDGE queue - the two rings drain in parallel and prefetch
            # ahead of the w / xin consumers
            for b in range(BC):
                xT_hi = pat.tile([128, N], BF, tag="xth")
                xT_lo = pat.tile([DA, N], BF, tag="xtl")
                nc.sync.dma_start(xT_hi[:, 0 : N // 2], xT_d[b, 0:128, 0 : N // 2])
                nc.sync.dma_start(xT_hi[:, N // 2 : N], xT_d[b, 0:128, N // 2 : N])
                nc.sync.dma_start(xT_lo[:], xT_d[b, 128:D, :])
                xts[b] = (xT_hi, xT_lo)
                xv_t = pa.tile([128, NT * DP], BF, tag="x")
                nc.scalar.dma_start(xv_t[:], x_d[b])
                xvs[b] = xv_t
            # routing matmuls + exp, one dense PE train
            for b in range(BC):
                xT_hi, xT_lo = xts[b]
                for c in range(NC):
                    wps = ppw.tile([E, 512], F32, tag="wt")
                    sl = slice(512 * c, 512 * (c + 1))
                    nc.tensor.matmul(
                        wps[:], phi_hi[:], xT_hi[:, sl], start=True, stop=False
                    )
                    nc.tensor.matmul(
                        wps[:], phi_lo[:], xT_lo[:, sl], start=False, stop=True
                    )
                    nc.scalar.activation(expCT[b][:, sl], wps[:], AF.Exp)
                # partition-move into the stack tile (rows E*(b%G)..)
                nc.gpsimd.dma_start(
                    expG[b // G][E * (b % G) : E * (b % G + 1), :], expCT[b][:]
                )
            # shared transposes: [G*E, 128] chunks -> [128, G*E]
            for g in range(NG):
                egv = expG[g][:].rearrange("q (p t) -> q t p", t=NT)
                for t2 in range(NT // 2):
                    trp = ppt.tile([128, 2 * G * E], F32, tag="trp")
                    for k in range(2):
                        nc.tensor.matmul(
                            trp[:, k * G * E : (k + 1) * G * E],
                            egv[:, 2 * t2 + k, :],
                            ident[0 : G * E, 0 : G * E],
                            start=True,
                            stop=True,
                        )
                    if t2 % 2 == 0:
                        nc.vector.tensor_copy(
                            ecG[g][:, 2 * G * E * t2 : 2 * G * E * (t2 + 1)],
                            trp[:],
                        )
                    else:
                        nc.scalar.copy(
                            ecG[g][:, 2 * G * E * t2 : 2 * G * E * (t2 + 1)],
                            trp[:],
                        )
            # xin per batch, dense
            for b in range(BC):
                g, bg = b // G, b % G
                xv = xvs[b]
                xps = ppx.tile([E, DP], F32, tag="xps")
                for t in range(NT):
                    nc.tensor.matmul(
                        xps[:],
                        ecG[g][:, G * E * t + E * bg : G * E * t + E * (bg + 1)],
                        xv[:].rearrange("p (t c) -> p t c", c=DP)[:, t, :],
                        start=(t == 0),
                        stop=(t == NT - 1),
                    )
                xin_sb = pa2.tile([E, DP], BF, tag="xsb")
                nc.scalar.copy(xin_sb[:], xps[:])
                # transpose into the L1 pack layout [d, (e, b)]
                xth = ppxt.tile([128, E], F32, tag="xh")
                xtl = ppxt.tile([DP - 128, E], F32, tag="xl")
                nc.tensor.matmul(
                    xth[:], xin_sb[:, 0:128], ident[0:E, 0:E], start=True, stop=True
                )
                nc.tensor.matmul(
                    xtl[:],
                    xin_sb[:, 128:DP],
                    ident[0:E, 0:E],
                    start=True,
                    stop=True,
                )
                xhv = xin_hi[:].rearrange("p (e w) -> p e w", w=BC)
                xlv = xin_lo[:].rearrange("p (e w) -> p e w", w=BC)
                nc.vector.tensor_copy(xhv[:, :, b], xth[:])
                nc.vector.tensor_copy(xlv[:, :, b], xtl[:])

        # ---------------- MLP phase (packed over (e, b) rows) ----------------
        def ln_mish(hs, pool, H, gr, ber):
            """LayerNorm + mish of SBUF [EB, H] f32 -> bf16."""
            s1 = pool.tile([EB, 1], F32, tag="s1")
            nc.vector.reduce_sum(s1[:], hs, axis=mybir.AxisListType.X)
            mean = pool.tile([EB, 1], F32, tag="mean")
            nc.scalar.mul(mean[:], s1[:], 1.0 / H)
            xc = pool.tile([EB, H], F32, tag="xc")
            nc.vector.tensor_scalar_sub(xc[:], hs, mean[:])
            sq = pool.tile([EB, H], F32, tag="sq")
            var = pool.tile([EB, 1], F32, tag="var")
            nc.scalar.activation(sq[:], xc[:], AF.Square, accum_out=var[:])
            std = pool.tile([EB, 1], F32, tag="std")
            nc.scalar.activation(
                std[:], var[:], AF.Sqrt, bias=eps_col[0:EB, :], scale=1.0 / H
            )
            rstd = pool.tile([EB, 1], F32, tag="rstd")
            nc.vector.reciprocal(rstd[:], std[:])
            xn = pool.tile([EB, H], F32, tag="xn")
            nc.vector.tensor_scalar_mul(xn[:], xc[:], rstd[:])
            if gr is not None:
                xg = pool.tile([EB, H], F32, tag="xg")
                nc.vector.tensor_mul(xg[:], xn[:], gr)
                xn = pool.tile([EB, H], F32, tag="xb")
                nc.vector.tensor_add(xn[:], xg[:], ber)
            # mish(x) = x * tanh(ln(1 + e^x))
            ex = pool.tile([EB, H], F32, tag="ex")
            nc.scalar.activation(ex[:], xn[:], AF.Exp)
            sp = pool.tile([EB, H], F32, tag="sp")
            nc.scalar.activation(sp[:], ex[:], AF.Ln, bias=1.0)
            th = pool.tile([EB, H], F32, tag="th")
            nc.scalar.activation(th[:], sp[:], AF.Tanh)
            hm = pool.tile([EB, H], BF, tag="hm")
            nc.vector.tensor_mul(hm[:], xn[:], th[:])
            return hm

        def transpose_pack(hm, pool, ppool, H, name):
            """[EB, H] bf16 -> hT [128, (H//128)*EB] via identity matmuls."""
            hT = pool.tile([128, (H // 128) * EB], BF, tag=name, name=name)
            for c in range(H // 128):
                ptp = ppool.tile([128, EB], F32, tag="mtr")
                nc.tensor.matmul(
                    ptp[:],
                    hm[:, 128 * c : 128 * (c + 1)],
                    ident[:, 0:EB],
                    start=True,
                    stop=True,
                )
                if c % 2 == 0:
                    nc.vector.tensor_copy(hT[:, c * EB : (c + 1) * EB], ptp[:])
                else:
                    nc.scalar.copy(hT[:, c * EB : (c + 1) * EB], ptp[:])
            return hT

        with tc.tile_pool(name="pm", bufs=1) as pm, tc.tile_pool(
            name="pm_st", bufs=3
        ) as pst, tc.tile_pool(name="pm_ps", bufs=3, space="PSUM") as pmps, tc.tile_pool(
            name="pm_ps_tr", bufs=2, space="PSUM"
        ) as pmpst, tc.tile_pool(name="pm_ps_eo", bufs=1, space="PSUM") as pmpse:
            # deferred w2 slabs (experts 8-15): SBUF freed by phase A pools,
            # DMA streams during L1/ln1
            for i in range(2, E // NW2):
                t = pm.tile([128, NW2 * C1 * H2], BF, tag=f"w2d{i}")
                nc.gpsimd.dma_start(
                    t[:], w2_d[:, i * NW2 * C1 * H2 : (i + 1) * NW2 * C1 * H2]
                )
                w2_sb.append(t)
            w1hv = w1h_sb[:].rearrange("p (e h) -> p e h", e=E)
            w1lv = w1l_sb[:].rearrange("p (e h) -> p e h", e=E)
            h1_all = pm.tile([EB, H1], F32, tag="h1_all", name="h1_all")
            for e in range(E):
                hp = pmps.tile([BC, H1], F32, tag="hp")
                nc.tensor.matmul(
                    hp[:],
                    xin_hi[:, e * BC : (e + 1) * BC],
                    w1hv[:, e, :],
                    start=True,
                    stop=False,
                )
                nc.tensor.matmul(
                    hp[:],
                    xin_lo[:, e * BC : (e + 1) * BC],
                    w1lv[:, e, :],
                    start=False,
                    stop=True,
                )
                hst = pst.tile([BC, H1], F32, tag="hst")
                if e % 2 == 0:
                    nc.vector.tensor_copy(hst[:], hp[:])
                else:
                    nc.scalar.copy(hst[:], hp[:])
                eng = nc.gpsimd if e % 2 == 0 else nc.sync
                eng.dma_start(h1_all[e * BC : (e + 1) * BC, :], hst[:])
            h1m = ln_mish(
                h1_all[:],
                pm,
                H1,
                g1sb[:] if has_g1 else None,
                be1sb[:] if has_g1 else None,
            )
            h1T = transpose_pack(h1m, pm, pmpst, H1, "h1T")

            h2_all = pm.tile([EB, H2], F32, tag="h2_all", name="h2_all")
            for e in range(E):
                w2v = w2_sb[e // NW2][:].rearrange(
                    "p (q c h) -> p q c h", q=NW2, c=C1
                )
                hp = pmps.tile([BC, H2], F32, tag="hp")
                for c1 in range(C1):
                    nc.tensor.matmul(
                        hp[:],
                        h1T[:, c1 * EB + e * BC : c1 * EB + (e + 1) * BC],
                        w2v[:, e % NW2, c1, :],
                        start=(c1 == 0),
                        stop=(c1 == C1 - 1 and not has_b2),
                    )
                if has_b2:
                    nc.tensor.matmul(
                        hp[:],
                        ones_sb[0:1, 0:BC],
                        b2sb[0:1, e * H2 : (e + 1) * H2],
                        start=False,
                        stop=True,
                    )
                hst = pst.tile([BC, H2], F32, tag="hst")
                if e % 2 == 0:
                    nc.vector.tensor_copy(hst[:], hp[:])
                else:
                    nc.scalar.copy(hst[:], hp[:])
                eng = nc.gpsimd if e % 2 == 0 else nc.sync
                eng.dma_start(h2_all[e * BC : (e + 1) * BC, :], hst[:])
            h2m = ln_mish(
                h2_all[:],
                pm,
                H2,
                g2sb[:] if has_g2 else None,
                be2sb[:] if has_g2 else None,
            )
            h2T = transpose_pack(h2m, pm, pmpst, H2, "h2T")

            # Layer 3 -> eops [DZ, (e, b)]
            w3v = w3_sb[:].rearrange("p (e c d) -> p e c d", e=E, c=C2)
            eops = pmpse.tile([128, EB], F32, tag="eot")
            for e in range(E):
                for c in range(C2):
                    nc.tensor.matmul(
                        eops[0:DZ, e * BC : (e + 1) * BC],
                        w3v[:, e, c, :],
                        h2T[:, c * EB + e * BC : c * EB + (e + 1) * BC],
                        start=(c == 0),
                        stop=(c == C2 - 1 and not has_b3),
                    )
                if has_b3:
                    nc.tensor.matmul(
                        eops[0:DZ, e * BC : (e + 1) * BC],
                        b3sb[0:1, e * DZ : (e + 1) * DZ],
                        ones_sb[0:1, 0:BC],
                        start=False,
                        stop=True,
                    )
            eot_sb = pm.tile([128, EB], BF, tag="eot_sb")
            nc.vector.tensor_copy(eot_sb[:], eops[:])
            # regroup into per-batch [E, DZ+2] via identity matmuls
            eotv = eot_sb[:].rearrange("p (e w) -> p w e", w=BC)
            for b in range(BC):
                peo = pmpst.tile([E, DZ], F32, tag="peo")
                nc.tensor.matmul(
                    peo[:], eotv[:, b, :], ident[:, 0:DZ], start=True, stop=True
                )
                nc.vector.tensor_copy(eo[b][:, 0:DZ], peo[:])

        # ---------------- Combine phase ----------------
        with tc.tile_pool(name="pc", bufs=4) as pc, tc.tile_pool(
            name="pc_st", bufs=2
        ) as pcst, tc.tile_pool(name="pc_ps", bufs=6, space="PSUM") as pcps:
            for b in range(BC):
                osb = pcst.tile([128, NT * DZ], F32, tag="osb")
                ov = osb[:].rearrange("p (t d) -> p t d", d=DZ)
                ecv = expCT[b][:].rearrange("e (p t) -> e t p", t=NT)
                # combine softmax denominators for all chunks at once, from
                # the token-major copy of the same bf16 exp values
                ecv4 = ecG[b // G][:].rearrange(
                    "p (t w e) -> p t w e", w=G, e=E
                )
                sn = pc.tile([128, NT], F32, tag="sn")
                nc.vector.reduce_sum(
                    sn[:], ecv4[:, :, b % G, :], axis=mybir.AxisListType.X
                )
                rna = pc.tile([128, NT], F32, tag="rna")
                nc.vector.reciprocal(rna[:], sn[:])
                for t in range(NT):
                    ops = pcps.tile([128, DZ], F32, tag="o")
                    nc.tensor.matmul(
                        ops[:], ecv[:, t, :], eo[b][:, :], start=True, stop=True
                    )
                    if t % 2 == 0:
                        nc.vector.tensor_scalar_mul(
                            ov[:, t, :], ops[:], rna[:, t : t + 1]
                        )
                    else:
                        nc.scalar.mul(ov[:, t, :], ops[:], rna[:, t : t + 1])
                nc.sync.dma_start(out_d[b], osb[:])
    return nc


# ---------------------------------------------------------------------------
# Host wrapper
# ---------------------------------------------------------------------------

_CACHE = {}


def _get_nc(key, *args):
    if key not in _CACHE:
        _CACHE[key] = build_nc(*args)
    return _CACHE[key]


def _prepare(z, a, phi, W1, b1, g1, be1, W2, b2, g2, be2, W3, b3):
    """Build (cached) the Bass program and per-core input maps."""
    import ml_dtypes

    BFNP = ml_dtypes.bfloat16

    z = np.asarray(z, np.float32)
    a = np.asarray(a, np.float32)
    phi = np.asarray(phi, np.float32)
    W1 = np.asarray(W1, np.float32)
    b1 = np.asarray(b1, np.float32)
    g1 = np.asarray(g1, np.float32)
    be1 = np.asarray(be1, np.float32)
    W2 = np.asarray(W2, np.float32)
    b2 = np.asarray(b2, np.float32)
    g2 = np.asarray(g2, np.float32)
    be2 = np.asarray(be2, np.float32)
    W3 = np.asarray(W3, np.float32)
    b3 = np.asarray(b3, np.float32)

    B, N, DZ = z.shape
    DA = a.shape[2]
    D = DZ + DA
    DP = D + 2
    E = W1.shape[0]
    H1 = W1.shape[2]
    H2 = W2.shape[2]
    C1 = H1 // 128
    C2 = H2 // 128
    BC = B // NCORES
    NT = N // 128

    has_b2 = bool(np.any(b2))
    has_b3 = bool(np.any(b3))
    has_g1 = not (np.all(g1 == 1.0) and np.all(be1 == 0.0))
    has_g2 = not (np.all(g2 == 1.0) and np.all(be2 == 0.0))

    key = (BC, N, DZ, DA, E, H1, H2, has_b2, has_b3, has_g1, has_g2)
    nc = _get_nc(key, *key)

    x_full = np.empty((B, N, DP), np.float32)
    x_full[:, :, 0:DZ] = z
    x_full[:, :, DZ:D] = a
    x_full[:, :, D:DP] = 1.0
    x_bf = x_full.astype(BFNP).reshape(B, 128, NT * DP)
    xT_bf = np.ascontiguousarray(
        x_full[:, :, 0:D].transpose(0, 2, 1)
    ).astype(BFNP)
    phi_bf = np.ascontiguousarray(phi.reshape(D, E)).astype(BFNP)
    w1aug = np.zeros((E, DP, H1), np.float32)
    w1aug[:, 0:D, :] = W1
    w1aug[:, D, :] = b1
    w1h = np.ascontiguousarray(
        w1aug[:, 0:128, :].transpose(1, 0, 2).reshape(128, E * H1)
    ).astype(BFNP)
    w1l = np.ascontiguousarray(
        w1aug[:, 128:DP, :].transpose(1, 0, 2).reshape(DP - 128, E * H1)
    ).astype(BFNP)
    w2p = np.ascontiguousarray(
        W2.reshape(E, C1, 128, H2).transpose(2, 0, 1, 3).reshape(128, E * C1 * H2)
    ).astype(BFNP)
    w3p = np.ascontiguousarray(
        W3.reshape(E, C2, 128, DZ).transpose(2, 0, 1, 3).reshape(128, E * C2 * DZ)
    ).astype(BFNP)
    ident_np = np.eye(128, dtype=np.float32).astype(BFNP)
    ones_np = np.ones((128, 128), np.float32).astype(BFNP)

    in_maps = []
    for i in range(NCORES):
        m = {
            "x": np.ascontiguousarray(x_bf[i * BC : (i + 1) * BC]),
            "xT": np.ascontiguousarray(xT_bf[i * BC : (i + 1) * BC]),
            "phi": phi_bf,
            "w1h": w1h,
            "w1l": w1l,
            "w2": w2p,
            "w3": w3p,
            "ident": ident_np,
            "ones": ones_np,
        }
        if has_b2:
            m["b2"] = b2.reshape(1, E * H2).astype(BFNP)
        if has_b3:
            m["b3"] = b3.reshape(1, E * DZ).astype(BFNP)
        if has_g1:
            m["g1r"] = np.ascontiguousarray(np.repeat(g1, BC, axis=0))
            m["be1r"] = np.ascontiguousarray(np.repeat(be1, BC, axis=0))
        if has_g2:
            m["g2r"] = np.ascontiguousarray(np.repeat(g2, BC, axis=0))
            m["be2r"] = np.ascontiguousarray(np.repeat(be2, BC, axis=0))
        in_maps.append(m)
    return nc, in_maps


def kernel(**inputs):
    nc, in_maps = _prepare(**inputs)

    from concourse.bass_utils import run_bass_kernel_spmd

    res = run_bass_kernel_spmd(nc, in_maps, list(range(NCORES)))
    B = inputs["z"].shape[0]
    N = inputs["z"].shape[1]
    DZ = inputs["z"].shape[2]
    return np.concatenate([r["out"] for r in res.results], axis=0).reshape(
        B, N, DZ
    )


# revision 29
# speedup vs baseline: 1.0362x; 1.0052x over previous
"""Trainium2 Bass kernel for nn_CenMoEDynamicsModel (MoE routing).

Contract: kernel(**inputs) takes FULL unsharded numpy inputs and returns the
FULL [64, 2048, 128] f32 output. Data-parallel over B across 8 NeuronCores
(8 batches/core), expert weights replicated (collectives measured ~67us
fixed cost here - too slow for expert-parallelism at this size).

Math (per batch b):
  x = [z|a]                       [N, D]     D = 192
  w = x @ phi                     [N, E]     E = 16
  dispatch = softmax_n(w); xin = dispatch^T @ x          [E, D]
  h = mish(LN(xin@W1+b1)); h = mish(LN(h@W2+b2)); EO = h@W3+b3   [E, DZ]
  combine = softmax_e(w); out = combine @ EO             [N, DZ]

Design notes (PE-instruction-count driven; ~160ns/instruction fixed cost
dominates over cycles at these shapes):
  - all DMA'd data bf16 with host-pre-arranged dense layouts (>=2KB
    contiguous per partition line); LN/mish internals + output f32.
  - softmaxes via unshifted exp; the dispatch denominator rides as two ones
    columns baked into x => an s row in xin; each pre-LN row is then
    uniformly scaled by s (b1 rides the s row of augmented W1), so LayerNorm
    scale-invariance makes explicit 1/s normalization unnecessary.
  - exp(w) is e-major per batch [E, N] (combine lhsT); token-major copies
    for the xin matmuls come from transposes of 4-batch stacks ([64, 128]
    chunks => 32 transposes/core instead of 128), expressed as regular bf16
    matmuls against identity slices (exact for bf16, cheap moving dim).
  - xin per (batch, chunk): one [16, 194] matmul (ec slice stationary,
    x chunk moving); 2 identity-matmul transposes per batch produce the
    [d, (e, b)] pack for layer 1.
  - combine denominator via ones-columns carried in the expert outputs,
    normalized during PSUM evacuation.
"""

import sys

import numpy as np

sys.path.insert(0, "/opt/trn_rl_repo")

from contextlib import ExitStack

import concourse.bass as bass
import concourse.tile as tile
from concourse import mybir

F32 = mybir.dt.float32
BF = mybir.dt.bfloat16
AF = mybir.ActivationFunctionType

LN_EPS = 1e-5
NCORES = 8


def _split_drain_and_barrier(self, tick_clock, wait_clock):
    """Replacement for TileContext._drain_and_barrier.

    The stock version attaches every outstanding semaphore wait to ONE tail
    Drain instruction; this walrus build's codegen rejects Drains with more
    than a couple of sync waits ("Too many sync wait commands"). Emit one
    single-wait Drain per logical proc instead (the SP queue executes them in
    order, so the final bare drain still happens after everything finished).
    """
    from concourse.vector_clock import ScopedClock, VectorClock

    nc = self.nc
    gc = tick_clock.global_clock
    n = len(gc)
    for i in range(n):
        t = gc[i]
        if t <= 0:
            continue
        v = VectorClock([0] * n)
        v.require_at_least(i, t)
        d = nc.sync.drain()
        wait_clock.add_sem_waits(d.ins, ScopedClock({None: v}))
    nc.sync.drain()
    nc.all_engine_barrier()
    assert self.sems is not None
    popped = nc._tile_sem_poison_stack.pop()
    assert popped is self._sem_poison
    nc.clear_and_free_semaphores(list(self.sems.allocated().values()))
    nc.all_engine_barrier()


tile.TileContext._drain_and_barrier = _split_drain_and_barrier

# This walrus build rejects instructions carrying more than a couple of sync
# waits ("Too many sync wait commands" in CoreV3 codegen), while Tile freely
# attaches 3+. Split excess waits onto NoOp carrier instructions (same engine
# queue, executed in order => semantics preserved) at BIR-serialization time.
_MAX_WAITS = 1


def _split_waits_json(bir: bytes) -> bytes:
    import orjson

    m = orjson.loads(bir)
    changed = False
    ctr = 0
    for f in m.get("functions", []):
        for b in f.get("blocks", []):
            out = []
            for i in b.get("instructions", []):
                si = i.get("sync_info")
                ow = (si or {}).get("on_wait") or []
                if len(ow) > _MAX_WAITS:
                    head = ow[: -_MAX_WAITS]
                    for j in range(0, len(head), _MAX_WAITS):
                        ctr += 1
                        out.append(
                            {
                                "debug": i.get("debug", 0),
                                "engine": i["engine"],
                                "ins": [],
                                "outs": [],
                                "name": f"{i['name']}-wsplit{ctr}",
                                "opcode": "NoOp",
                                "sync_info": {
                                    "on_wait": head[j : j + _MAX_WAITS],
                                    "on_update": [],
                                },
                            }
                        )
                    si["on_wait"] = ow[-_MAX_WAITS:]
                    changed = True
                out.append(i)
            b["instructions"] = out
    return orjson.dumps(m) if changed else bir


_orig_to_json_bytes = bass.Bass.to_json_bytes


def _patched_to_json_bytes(self):
    return _split_waits_json(_orig_to_json_bytes(self))


bass.Bass.to_json_bytes = _patched_to_json_bytes




def build_nc(BC, N, DZ, DA, E, H1, H2, has_b2, has_b3, has_g1, has_g2):
    """Build the per-core Bass program.

    BC batches per core. Requires DZ == 128, N % 512 == 0, H1 % 128 == 0,
    H2 % 128 == 0, E * BC == 128, E <= 16.
    """
    D = DZ + DA  # 192
    DP = D + 2  # 194 (two ones columns)
    NT = N // 128  # 16
    NC = N // 512  # 4
    EB = E * BC  # 128
    C1 = H1 // 128
    C2 = H2 // 128
    G = 4  # batches per transpose stack
    NG = BC // G
    assert DZ == 128 and DA < 128 and EB == 128 and N % 512 == 0 and E <= 16

    nc = bass.Bass()

    # ---- DRAM tensors: all host-pre-arranged for dense [<=128, F] DMAs ----
    x_d = nc.dram_tensor("x", [BC, 128, NT * DP], BF, kind="ExternalInput")
    xT_d = nc.dram_tensor("xT", [BC, D, N], BF, kind="ExternalInput")
    phi_d = nc.dram_tensor("phi", [D, E], BF, kind="ExternalInput")
    w1h_d = nc.dram_tensor("w1h", [128, E * H1], BF, kind="ExternalInput")
    w1l_d = nc.dram_tensor(
        "w1l", [DP - 128, E * H1], BF, kind="ExternalInput"
    )
    w2_d = nc.dram_tensor("w2", [128, E * C1 * H2], BF, kind="ExternalInput")
    w3_d = nc.dram_tensor("w3", [128, E * C2 * DZ], BF, kind="ExternalInput")
    ident_d = nc.dram_tensor("ident", [128, 128], BF, kind="ExternalInput")
    ones_d = nc.dram_tensor("ones", [128, 128], BF, kind="ExternalInput")
    if has_b2:
        b2_d = nc.dram_tensor("b2", [1, E * H2], BF, kind="ExternalInput")
    if has_b3:
        b3_d = nc.dram_tensor("b3", [1, E * DZ], BF, kind="ExternalInput")
    if has_g1:
        g1_d = nc.dram_tensor("g1r", [EB, H1], F32, kind="ExternalInput")
        be1_d = nc.dram_tensor("be1r", [EB, H1], F32, kind="ExternalInput")
    if has_g2:
        g2_d = nc.dram_tensor("g2r", [EB, H2], F32, kind="ExternalInput")
        be2_d = nc.dram_tensor("be2r", [EB, H2], F32, kind="ExternalInput")
    out_d = nc.dram_tensor(
        "out", [BC, 128, NT * DZ], F32, kind="ExternalOutput"
    )

    with tile.TileContext(nc) as tc, ExitStack() as ctx:
        perm = ctx.enter_context(tc.tile_pool(name="perm", bufs=1))
        ident = perm.tile([128, 128], BF)
        ones_sb = perm.tile([128, 128], BF)
        phi_hi = perm.tile([128, E], BF)
        phi_lo = perm.tile([DA, E], BF)
        nc.sync.dma_start(ident[:], ident_d[:, :])
        nc.sync.dma_start(ones_sb[:], ones_d[:, :])
        nc.sync.dma_start(phi_hi[:], phi_d[0:128, :])
        nc.sync.dma_start(phi_lo[:], phi_d[128:D, :])

        # Weight loads: issued up front on the gpsimd (SWDGE) queue so they
        # stream during phase A.
        w1h_sb = perm.tile([128, E * H1], BF, name="w1h_sb")
        w1l_sb = perm.tile([DP - 128, E * H1], BF, name="w1l_sb")
        w3_sb = perm.tile([128, E * C2 * DZ], BF, name="w3_sb")
        NW2 = 4  # experts per w2 slab
        # slabs 0-1 preloaded during phase A; slabs 2-3 allocated in the MLP
        # pool (their SBUF overlaps phase A's x tiles) and stream during L1
        w2_sb = [
            perm.tile([128, NW2 * C1 * H2], BF, name=f"w2_sb{i}")
            for i in range(2)
        ]
        def _load_weights():
            # emitted AFTER the x loads: weights ride the same HWDGE queues
            # behind the x stream, landing during the trs/xin stretch. The
            # gpsimd queue stays free for the latency-critical expG moves.
            nc.sync.dma_start(w1h_sb[:], w1h_d[:, :])
            nc.scalar.dma_start(w1l_sb[:], w1l_d[:, :])
            nc.sync.dma_start(w3_sb[:], w3_d[:, :])
            for i in range(2):
                eng = nc.scalar if i % 2 == 0 else nc.sync
                eng.dma_start(
                    w2_sb[i][:],
                    w2_d[:, i * NW2 * C1 * H2 : (i + 1) * NW2 * C1 * H2],
                )
        if has_b2:
            b2sb = perm.tile([1, E * H2], BF)
            nc.gpsimd.dma_start(b2sb[:], b2_d[:, :])
        if has_b3:
            b3sb = perm.tile([1, E * DZ], BF)
            nc.gpsimd.dma_start(b3sb[:], b3_d[:, :])
        g1sb = be1sb = g2sb = be2sb = None
        if has_g1:
            g1sb = perm.tile([EB, H1], F32)
            be1sb = perm.tile([EB, H1], F32)
            nc.gpsimd.dma_start(g1sb[:], g1_d[:, :])
            nc.gpsimd.dma_start(be1sb[:], be1_d[:, :])
        if has_g2:
            g2sb = perm.tile([EB, H2], F32)
            be2sb = perm.tile([EB, H2], F32)
            nc.gpsimd.dma_start(g2sb[:], g2_d[:, :])
            nc.gpsimd.dma_start(be2sb[:], be2_d[:, :])

        # exp(w) e-major per batch (combine lhsT; partitions 0..E-1)
        expCT = [
            perm.tile([E, N], BF, tag=f"expCT{b}", name=f"expCT{b}")
            for b in range(BC)
        ]
        # 4-batch stacks for the shared transposes (partitions 0..G*E-1)
        expG = [
            perm.tile([G * E, N], BF, tag=f"expG{g}", name=f"expG{g}")
            for g in range(NG)
        ]
        # token-major exp(w) per stack: [tok, (t, b in stack, e)]
        ecG = [
            perm.tile([128, NT * G * E], BF, tag=f"ecG{g}", name=f"ecG{g}")
            for g in range(NG)
        ]
        # xin pack [d, (e, b)]; lo rows DA..DA+1 hold s_e
        xin_hi = perm.tile([128, EB], BF)
        xin_lo = perm.tile([DP - 128, EB], BF)
        # expert outputs per batch [E, DZ]
        eo = [
            perm.tile([E, DZ], BF, tag=f"eo{b}", name=f"eo{b}")
            for b in range(BC)
        ]
        eps_col = perm.tile([128, 1], F32)
        nc.vector.memset(eps_col[:], LN_EPS)

        # ---------------- Phase A: routing + xin ----------------
        with tc.tile_pool(name="pa", bufs=8) as pa, tc.tile_pool(
            name="pat", bufs=3
        ) as pat, tc.tile_pool(name="pa2", bufs=2) as pa2, tc.tile_pool(
            name="pa_ps_w", bufs=2, space="PSUM"
        ) as ppw, tc.tile_pool(
            name="pa_ps_tr", bufs=2, space="PSUM"
        ) as ppt, tc.tile_pool(
            name="pa_ps_xin", bufs=2, space="PSUM"
        ) as ppx, tc.tile_pool(name="pa_ps_xt", bufs=1, space="PSUM") as ppxt:
            xvs = {}
            xts = {}
            # all loads first: xT on the sync HWDGE queue, xv on the Act
            # HWDGE queue - the two rings drain in parallel and prefetch
            # ahead of the w / xin consumers
            for b in range(BC):
                xT_hi = pat.tile([128, N], BF, tag="xth")
                xT_lo = pat.tile([DA, N], BF, tag="xtl")
                nc.sync.dma_start(xT_hi[:, 0 : N // 2], xT_d[b, 0:128, 0 : N // 2])
                nc.sync.dma_start(xT_hi[:, N // 2 : N], xT_d[b, 0:128, N // 2 : N])
                nc.sync.dma_start(xT_lo[:], xT_d[b, 128:D, :])
                xts[b] = (xT_hi, xT_lo)
                xv_t = pa.tile([128, NT * DP], BF, tag="x")
                nc.scalar.dma_start(xv_t[:], x_d[b])
                xvs[b] = xv_t
            _load_weights()
            # routing matmuls + exp, one dense PE train
            for b in range(BC):
                xT_hi, xT_lo = xts[b]
                for c in range(NC):
                    wps = ppw.tile([E, 512], F32, tag="wt")
                    sl = slice(512 * c, 512 * (c + 1))
                    nc.tensor.matmul(
                        wps[:], phi_hi[:], xT_hi[:, sl], start=True, stop=False
                    )
                    nc.tensor.matmul(
                        wps[:], phi_lo[:], xT_lo[:, sl], start=False, stop=True
                    )
                    nc.scalar.activation(expCT[b][:, sl], wps[:], AF.Exp)
                # partition-move into the stack tile (rows E*(b%G)..)
                nc.gpsimd.dma_start(
                    expG[b // G][E * (b % G) : E * (b % G + 1), :], expCT[b][:]
                )
            # shared transposes: [G*E, 128] chunks -> [128, G*E]
            for g in range(NG):
                egv = expG[g][:].rearrange("q (p t) -> q t p", t=NT)
                for t2 in range(NT // 2):
                    trp = ppt.tile([128, 2 * G * E], F32, tag="trp")
                    for k in range(2):
                        nc.tensor.matmul(
                            trp[:, k * G * E : (k + 1) * G * E],
                            egv[:, 2 * t2 + k, :],
                            ident[0 : G * E, 0 : G * E],
                            start=True,
                            stop=True,
                        )
                    if t2 % 2 == 0:
                        nc.vector.tensor_copy(
                            ecG[g][:, 2 * G * E * t2 : 2 * G * E * (t2 + 1)],
                            trp[:],
                        )
                    else:
                        nc.scalar.copy(
                            ecG[g][:, 2 * G * E * t2 : 2 * G * E * (t2 + 1)],
                            trp[:],
                        )
            # xin per batch, dense
            for b in range(BC):
                g, bg = b // G, b % G
                xv = xvs[b]
                xps = ppx.tile([E, DP], F32, tag="xps")
                for t in range(NT):
                    nc.tensor.matmul(
                        xps[:],
                        ecG[g][:, G * E * t + E * bg : G * E * t + E * (bg + 1)],
                        xv[:].rearrange("p (t c) -> p t c", c=DP)[:, t, :],
                        start=(t == 0),
                        stop=(t == NT - 1),
                    )
                xin_sb = pa2.tile([E, DP], BF, tag="xsb")
                nc.scalar.copy(xin_sb[:], xps[:])
                # transpose into the L1 pack layout [d, (e, b)]
                xth = ppxt.tile([128, E], F32, tag="xh")
                xtl = ppxt.tile([DP - 128, E], F32, tag="xl")
                nc.tensor.matmul(
                    xth[:], xin_sb[:, 0:128], ident[0:E, 0:E], start=True, stop=True
                )
                nc.tensor.matmul(
                    xtl[:],
                    xin_sb[:, 128:DP],
                    ident[0:E, 0:E],
                    start=True,
                    stop=True,
                )
                xhv = xin_hi[:].rearrange("p (e w) -> p e w", w=BC)
                xlv = xin_lo[:].rearrange("p (e w) -> p e w", w=BC)
                nc.vector.tensor_copy(xhv[:, :, b], xth[:])
                nc.vector.tensor_copy(xlv[:, :, b], xtl[:])

        # ---------------- MLP phase (packed over (e, b) rows) ----------------
        def ln_mish(hs, pool, H, gr, ber):
            """LayerNorm + mish of SBUF [EB, H] f32 -> bf16."""
            s1 = pool.tile([EB, 1], F32, tag="s1")
            nc.vector.reduce_sum(s1[:], hs, axis=mybir.AxisListType.X)
            mean = pool.tile([EB, 1], F32, tag="mean")
            nc.scalar.mul(mean[:], s1[:], 1.0 / H)
            xc = pool.tile([EB, H], F32, tag="xc")
            nc.vector.tensor_scalar_sub(xc[:], hs, mean[:])
            sq = pool.tile([EB, H], F32, tag="sq")
            var = pool.tile([EB, 1], F32, tag="var")
            nc.scalar.activation(sq[:], xc[:], AF.Square, accum_out=var[:])
            std = pool.tile([EB, 1], F32, tag="std")
            nc.scalar.activation(
                std[:], var[:], AF.Sqrt, bias=eps_col[0:EB, :], scale=1.0 / H
            )
            rstd = pool.tile([EB, 1], F32, tag="rstd")
            nc.vector.reciprocal(rstd[:], std[:])
            xn = pool.tile([EB, H], F32, tag="xn")
            nc.vector.tensor_scalar_mul(xn[:], xc[:], rstd[:])
            if gr is not None:
                xg = pool.tile([EB, H], F32, tag="xg")
                nc.vector.tensor_mul(xg[:], xn[:], gr)
                xn = pool.tile([EB, H], F32, tag="xb")
                nc.vector.tensor_add(xn[:], xg[:], ber)
            # mish(x) = x * tanh(ln(1 + e^x))
            ex = pool.tile([EB, H], F32, tag="ex")
            nc.scalar.activation(ex[:], xn[:], AF.Exp)
            sp = pool.tile([EB, H], F32, tag="sp")
            nc.scalar.activation(sp[:], ex[:], AF.Ln, bias=1.0)
            th = pool.tile([EB, H], F32, tag="th")
            nc.scalar.activation(th[:], sp[:], AF.Tanh)
            hm = pool.tile([EB, H], BF, tag="hm")
            nc.vector.tensor_mul(hm[:], xn[:], th[:])
            return hm

        def transpose_pack(hm, pool, ppool, H, name):
            """[EB, H] bf16 -> hT [128, (H//128)*EB] via identity matmuls."""
            hT = pool.tile([128, (H // 128) * EB], BF, tag=name, name=name)
            for c in range(H // 128):
                ptp = ppool.tile([128, EB], F32, tag="mtr")
                nc.tensor.matmul(
                    ptp[:],
                    hm[:, 128 * c : 128 * (c + 1)],
                    ident[:, 0:EB],
                    start=True,
                    stop=True,
                )
                if c % 2 == 0:
                    nc.vector.tensor_copy(hT[:, c * EB : (c + 1) * EB], ptp[:])
                else:
                    nc.scalar.copy(hT[:, c * EB : (c + 1) * EB], ptp[:])
            return hT

        with tc.tile_pool(name="pm", bufs=1) as pm, tc.tile_pool(
            name="pm_st", bufs=3
        ) as pst, tc.tile_pool(name="pm_ps", bufs=3, space="PSUM") as pmps, tc.tile_pool(
            name="pm_ps_tr", bufs=2, space="PSUM"
        ) as pmpst, tc.tile_pool(name="pm_ps_eo", bufs=1, space="PSUM") as pmpse:
            # deferred w2 slabs (experts 8-15): SBUF freed by phase A pools,
            # DMA streams during L1/ln1
            for i in range(2, E // NW2):
                t = pm.tile([128, NW2 * C1 * H2], BF, tag=f"w2d{i}")
                eng = nc.scalar if i % 2 == 0 else nc.sync
                eng.dma_start(
                    t[:], w2_d[:, i * NW2 * C1 * H2 : (i + 1) * NW2 * C1 * H2]
                )
                w2_sb.append(t)
            w1hv = w1h_sb[:].rearrange("p (e h) -> p e h", e=E)
            w1lv = w1l_sb[:].rearrange("p (e h) -> p e h", e=E)
            h1_all = pm.tile([EB, H1], F32, tag="h1_all", name="h1_all")
            for e in range(E):
                hp = pmps.tile([BC, H1], F32, tag="hp")
                nc.tensor.matmul(
                    hp[:],
                    xin_hi[:, e * BC : (e + 1) * BC],
                    w1hv[:, e, :],
                    start=True,
                    stop=False,
                )
                nc.tensor.matmul(
                    hp[:],
                    xin_lo[:, e * BC : (e + 1) * BC],
                    w1lv[:, e, :],
                    start=False,
                    stop=True,
                )
                hst = pst.tile([BC, H1], F32, tag="hst")
                if e % 2 == 0:
                    nc.vector.tensor_copy(hst[:], hp[:])
                else:
                    nc.scalar.copy(hst[:], hp[:])
                eng = nc.gpsimd if e % 2 == 0 else nc.sync
                eng.dma_start(h1_all[e * BC : (e + 1) * BC, :], hst[:])
            h1m = ln_mish(
                h1_all[:],
                pm,
                H1,
                g1sb[:] if has_g1 else None,
                be1sb[:] if has_g1 else None,
            )
            h1T = transpose_pack(h1m, pm, pmpst, H1, "h1T")

            h2_all = pm.tile([EB, H2], F32, tag="h2_all", name="h2_all")
            for e in range(E):
                w2v = w2_sb[e // NW2][:].rearrange(
                    "p (q c h) -> p q c h", q=NW2, c=C1
                )
                hp = pmps.tile([BC, H2], F32, tag="hp")
                for c1 in range(C1):
                    nc.tensor.matmul(
                        hp[:],
                        h1T[:, c1 * EB + e * BC : c1 * EB + (e + 1) * BC],
                        w2v[:, e % NW2, c1, :],
                        start=(c1 == 0),
                        stop=(c1 == C1 - 1 and not has_b2),
                    )
                if has_b2:
                    nc.tensor.matmul(
                        hp[:],
                        ones_sb[0:1, 0:BC],
                        b2sb[0:1, e * H2 : (e + 1) * H2],
                        start=False,
                        stop=True,
                    )
                hst = pst.tile([BC, H2], F32, tag="hst")
                if e % 2 == 0:
                    nc.vector.tensor_copy(hst[:], hp[:])
                else:
                    nc.scalar.copy(hst[:], hp[:])
                eng = nc.gpsimd if e % 2 == 0 else nc.sync
                eng.dma_start(h2_all[e * BC : (e + 1) * BC, :], hst[:])
            h2m = ln_mish(
                h2_all[:],
                pm,
                H2,
                g2sb[:] if has_g2 else None,
                be2sb[:] if has_g2 else None,
            )
            h2T = transpose_pack(h2m, pm, pmpst, H2, "h2T")

            # Layer 3 -> eops [DZ, (e, b)]
            w3v = w3_sb[:].rearrange("p (e c d) -> p e c d", e=E, c=C2)
            eops = pmpse.tile([128, EB], F32, tag="eot")
            for e in range(E):
                for c in range(C2):
                    nc.tensor.matmul(
                        eops[0:DZ, e * BC : (e + 1) * BC],
                        w3v[:, e, c, :],
                        h2T[:, c * EB + e * BC : c * EB + (e + 1) * BC],
                        start=(c == 0),
                        stop=(c == C2 - 1 and not has_b3),
                    )
                if has_b3:
                    nc.tensor.matmul(
                        eops[0:DZ, e * BC : (e + 1) * BC],
                        b3sb[0:1, e * DZ : (e + 1) * DZ],
                        ones_sb[0:1, 0:BC],
                        start=False,
                        stop=True,
                    )
            eot_sb = pm.tile([128, EB], BF, tag="eot_sb")
            nc.vector.tensor_copy(eot_sb[:], eops[:])
            # regroup into per-batch [E, DZ+2] via identity matmuls
            eotv = eot_sb[:].rearrange("p (e w) -> p w e", w=BC)
            for b in range(BC):
                peo = pmpst.tile([E, DZ], F32, tag="peo")
                nc.tensor.matmul(
                    peo[:], eotv[:, b, :], ident[:, 0:DZ], start=True, stop=True
                )
                nc.vector.tensor_copy(eo[b][:, 0:DZ], peo[:])

        # ---------------- Combine phase ----------------
        with tc.tile_pool(name="pc", bufs=4) as pc, tc.tile_pool(
            name="pc_st", bufs=2
        ) as pcst, tc.tile_pool(name="pc_ps", bufs=6, space="PSUM") as pcps:
            for b in range(BC):
                osb = pcst.tile([128, NT * DZ], F32, tag="osb")
                ov = osb[:].rearrange("p (t d) -> p t d", d=DZ)
                ecv = expCT[b][:].rearrange("e (p t) -> e t p", t=NT)
                # combine softmax denominators for all chunks at once, from
                # the token-major copy of the same bf16 exp values
                ecv4 = ecG[b // G][:].rearrange(
                    "p (t w e) -> p t w e", w=G, e=E
                )
                sn = pc.tile([128, NT], F32, tag="sn")
                nc.vector.reduce_sum(
                    sn[:], ecv4[:, :, b % G, :], axis=mybir.AxisListType.X
                )
                rna = pc.tile([128, NT], F32, tag="rna")
                nc.vector.reciprocal(rna[:], sn[:])
                for t in range(NT):
                    ops = pcps.tile([128, DZ], F32, tag="o")
                    nc.tensor.matmul(
                        ops[:], ecv[:, t, :], eo[b][:, :], start=True, stop=True
                    )
                    if t % 2 == 0:
                        nc.vector.tensor_scalar_mul(
                            ov[:, t, :], ops[:], rna[:, t : t + 1]
                        )
                    else:
                        nc.scalar.mul(ov[:, t, :], ops[:], rna[:, t : t + 1])
                nc.sync.dma_start(out_d[b], osb[:])
    return nc


# ---------------------------------------------------------------------------
# Host wrapper
# ---------------------------------------------------------------------------

_CACHE = {}


def _get_nc(key, *args):
    if key not in _CACHE:
        _CACHE[key] = build_nc(*args)
    return _CACHE[key]


def _prepare(z, a, phi, W1, b1, g1, be1, W2, b2, g2, be2, W3, b3):
    """Build (cached) the Bass program and per-core input maps."""
    import ml_dtypes

    BFNP = ml_dtypes.bfloat16

    z = np.asarray(z, np.float32)
    a = np.asarray(a, np.float32)
    phi = np.asarray(phi, np.float32)
    W1 = np.asarray(W1, np.float32)
    b1 = np.asarray(b1, np.float32)
    g1 = np.asarray(g1, np.float32)
    be1 = np.asarray(be1, np.float32)
    W2 = np.asarray(W2, np.float32)
    b2 = np.asarray(b2, np.float32)
    g2 = np.asarray(g2, np.float32)
    be2 = np.asarray(be2, np.float32)
    W3 = np.asarray(W3, np.float32)
    b3 = np.asarray(b3, np.float32)

    B, N, DZ = z.shape
    DA = a.shape[2]
    D = DZ + DA
    DP = D + 2
    E = W1.shape[0]
    H1 = W1.shape[2]
    H2 = W2.shape[2]
    C1 = H1 // 128
    C2 = H2 // 128
    BC = B // NCORES
    NT = N // 128

    has_b2 = bool(np.any(b2))
    has_b3 = bool(np.any(b3))
    has_g1 = not (np.all(g1 == 1.0) and np.all(be1 == 0.0))
    has_g2 = not (np.all(g2 == 1.0) and np.all(be2 == 0.0))

    key = (BC, N, DZ, DA, E, H1, H2, has_b2, has_b3, has_g1, has_g2)
    nc = _get_nc(key, *key)

    x_full = np.empty((B, N, DP), np.float32)
    x_full[:, :, 0:DZ] = z
    x_full[:, :, DZ:D] = a
    x_full[:, :, D:DP] = 1.0
    x_bf = x_full.astype(BFNP).reshape(B, 128, NT * DP)
    xT_bf = np.ascontiguousarray(
        x_full[:, :, 0:D].transpose(0, 2, 1)
    ).astype(BFNP)
    phi_bf = np.ascontiguousarray(phi.reshape(D, E)).astype(BFNP)
    w1aug = np.zeros((E, DP, H1), np.float32)
    w1aug[:, 0:D, :] = W1
    w1aug[:, D, :] = b1
    w1h = np.ascontiguousarray(
        w1aug[:, 0:128, :].transpose(1, 0, 2).reshape(128, E * H1)
    ).astype(BFNP)
    w1l = np.ascontiguousarray(
        w1aug[:, 128:DP, :].transpose(1, 0, 2).reshape(DP - 128, E * H1)
    ).astype(BFNP)
    w2p = np.ascontiguousarray(
        W2.reshape(E, C1, 128, H2).transpose(2, 0, 1, 3).reshape(128, E * C1 * H2)
    ).astype(BFNP)
    w3p = np.ascontiguousarray(
        W3.reshape(E, C2, 128, DZ).transpose(2, 0, 1, 3).reshape(128, E * C2 * DZ)
    ).astype(BFNP)
    ident_np = np.eye(128, dtype=np.float32).astype(BFNP)
    ones_np = np.ones((128, 128), np.float32).astype(BFNP)

    in_maps = []
    for i in range(NCORES):
        m = {
            "x": np.ascontiguousarray(x_bf[i * BC : (i + 1) * BC]),
            "xT": np.ascontiguousarray(xT_bf[i * BC : (i + 1) * BC]),
            "phi": phi_bf,
            "w1h": w1h,
            "w1l": w1l,
            "w2": w2p,
            "w3": w3p,
            "ident": ident_np,
            "ones": ones_np,
        }
        if has_b2:
            m["b2"] = b2.reshape(1, E * H2).astype(BFNP)
        if has_b3:
            m["b3"] = b3.reshape(1, E * DZ).astype(BFNP)
        if has_g1:
            m["g1r"] = np.ascontiguousarray(np.repeat(g1, BC, axis=0))
            m["be1r"] = np.ascontiguousarray(np.repeat(be1, BC, axis=0))
        if has_g2:
            m["g2r"] = np.ascontiguousarray(np.repeat(g2, BC, axis=0))
            m["be2r"] = np.ascontiguousarray(np.repeat(be2, BC, axis=0))
        in_maps.append(m)
    return nc, in_maps


def kernel(**inputs):
    nc, in_maps = _prepare(**inputs)

    from concourse.bass_utils import run_bass_kernel_spmd

    res = run_bass_kernel_spmd(nc, in_maps, list(range(NCORES)))
    B = inputs["z"].shape[0]
    N = inputs["z"].shape[1]
    DZ = inputs["z"].shape[2]
    return np.concatenate([r["out"] for r in res.results], axis=0).reshape(
        B, N, DZ
    )
